# revision 7
# baseline (speedup 1.0000x reference)
"""Trainium2 Bass kernel for nn_CognitiveRouter (hierarchical MoE routing).

Computation (see reference):
    module_logits = h @ Wm.T      (T,4)
    expert_logits = h @ We.T      (T,16)
    module_probs  = softmax(module_logits)
    expert_probs  = softmax(expert_logits.reshape(T,4,4), axis=-1)
    combined      = (module_probs[:,:,None]*expert_probs).reshape(T,16)
    topw, topi    = top_k(combined, 4);  topw /= (sum(topw)+1e-8)

Strategy:
  - Data-parallel: shard T=32768 across 8 NeuronCores (4096 tokens each).
  - Host prep: W = [Wm;We] (20,1536) and h are split into bf16 hi/lo pairs
    (x = hi + lo captures ~2^-18 relative accuracy, fp32-class logits) and
    transposed so the contraction dim D lands on SBUF partitions.
    (h + lo)@(Whi + Wlo) is computed with 2 matmul passes per k-tile into
    one PSUM accumulator of 40 columns [hi·Whi + lo·Whi | hi·Wlo + lo·Wlo],
    then logits = psum[:, :20] + psum[:, 20:40]  (all 4 cross terms).
  - Per core: 4 quarters of 1024 tokens; each quarter loads [128,12,1024]
    bf16 hi/lo slabs (3 MB DMAs), runs 8 token-tiles x 24 accumulating
    matmuls (stationary = h tile, moving = W 40 cols), then a packed
    softmax + top-k epilogue on [128, 8, 20] tiles. Top-4 via vector.max
    (top-8 sorted desc) + max_index (ties -> ascending distinct indices,
    matching jax.lax.top_k).
"""

import sys

if "/opt/trn_rl_repo" not in sys.path:
    sys.path.insert(0, "/opt/trn_rl_repo")

import ml_dtypes
import numpy as np

import concourse.bacc as bacc
import concourse.mybir as mybir
import concourse.tile as tile
from concourse.bass_utils import run_bass_kernel_spmd

N_CORES = 8
T, D = 32768, 1536
TLOC = T // N_CORES          # 4096 tokens per core
NQ = 8                       # token chunks per core
QT = TLOC // NQ              # 512 tokens per chunk
NTT = QT // 128              # 4 token tiles (of 128) per chunk
NKT = D // 128               # 12 k-tiles
NKH = NKT // 2               # k-tiles per DMA half-slab
NE = 20                      # 4 module + 16 expert logit columns

BF16 = mybir.dt.bfloat16
F32 = mybir.dt.float32
U32 = mybir.dt.uint32
AX = mybir.AxisListType.X
EXP = mybir.ActivationFunctionType.Exp

_CACHE = {}
LAST_RESULT = None  # BassKernelResults of the most recent run (for profiling)


def _build():
    nc = bacc.Bacc(trn_type="TRN2", target_bir_lowering=False, debug=False)

    hiT = nc.dram_tensor("hiT", [D, TLOC], BF16, kind="ExternalInput")
    loT = nc.dram_tensor("loT", [D, TLOC], BF16, kind="ExternalInput")
    wT = nc.dram_tensor("wT", [D, 2 * NE], BF16, kind="ExternalInput")
    o_probs = nc.dram_tensor("o_probs", [TLOC, 16], F32, kind="ExternalOutput")
    o_topw = nc.dram_tensor("o_topw", [TLOC, 4], F32, kind="ExternalOutput")
    o_topi = nc.dram_tensor("o_topi", [TLOC, 4], U32, kind="ExternalOutput")

    # DRAM views with the 128-partition dim innermost on the left
    hiT_v = hiT.ap().rearrange("(k p) t -> p k t", p=128)   # [128, 12, 4096]
    loT_v = loT.ap().rearrange("(k p) t -> p k t", p=128)
    wT_v = wT.ap().rearrange("(k p) e -> p k e", p=128)     # [128, 12, 40]
    # token t_local = q*QT + i*128 + p
    probs_v = o_probs.ap().rearrange("(q i p) e -> q p i e", p=128, i=NTT)
    topw_v = o_topw.ap().rearrange("(q i p) e -> q p i e", p=128, i=NTT)
    topi_v = o_topi.ap().rearrange("(q i p) e -> q p i e", p=128, i=NTT)

    with tile.TileContext(nc) as tc:
        with (
            tc.tile_pool(name="kt", bufs=5) as ktp,
            tc.tile_pool(name="wp", bufs=1) as wp,
            tc.tile_pool(name="ps", bufs=8, space="PSUM") as pp,
            tc.tile_pool(name="ep", bufs=2) as ep,
            tc.tile_pool(name="outp", bufs=3) as outp,
        ):
            w_sb = wp.tile([128, NKT, 2 * NE], BF16)
            nc.sync.dma_start(out=w_sb, in_=wT_v)

            for q in range(NQ):
                # two half-slabs per chunk so the first half's matmuls can
                # run while the second half is still streaming in
                slabs = []
                for h in range(2):
                    hi_sb = ktp.tile([128, NKH, QT], BF16, tag=f"hi{h}")
                    lo_sb = ktp.tile([128, NKH, QT], BF16, tag=f"lo{h}")
                    ks = slice(h * NKH, (h + 1) * NKH)
                    ts = slice(q * QT, (q + 1) * QT)
                    nc.sync.dma_start(out=hi_sb, in_=hiT_v[:, ks, ts])
                    nc.sync.dma_start(out=lo_sb, in_=loT_v[:, ks, ts])
                    slabs.append((hi_sb, lo_sb))

                # ---- matmuls: per token tile, 24 accumulating matmuls ----
                ps_list = [
                    pp.tile([128, 2 * NE], F32, name="ps", tag="ps")
                    for _ in range(NTT)
                ]
                for h, (hi_sb, lo_sb) in enumerate(slabs):
                    for i in range(NTT):
                        ps = ps_list[i]
                        for kk in range(NKH):
                            for si, src in enumerate((hi_sb, lo_sb)):
                                nc.tensor.matmul(
                                    ps,
                                    lhsT=src[:, kk, i * 128:(i + 1) * 128],
                                    rhs=w_sb[:, h * NKH + kk, :],
                                    start=(h == 0 and kk == 0 and si == 0),
                                    stop=(h == 1 and kk == NKH - 1 and si == 1),
                                )

                # ---- epilogue: logits -> hierarchical softmax -> top-4 ----
                # PSUM -> SBUF on ScalarE (one PSUM read port; DVE cannot read
                # two PSUM operands), then a single SBUF-only add folds the
                # [hi*Whi+lo*Whi | hi*Wlo+lo*Wlo] halves.
                ps_sb = ep.tile([128, NTT, 2 * NE], F32, tag="ps_sb")
                for i in range(NTT):
                    nc.scalar.activation(
                        ps_sb[:, i, :], ps_list[i], mybir.ActivationFunctionType.Copy
                    )
                logits = ep.tile([128, NTT, NE], F32, tag="logits")
                nc.vector.tensor_add(
                    logits, ps_sb[:, :, 0:NE], ps_sb[:, :, NE:2 * NE]
                )

                lg_m = logits[:, :, 0:4]                                   # [128,8,4]
                lg_e = logits[:, :, 4:NE].rearrange("p g (m e) -> p g m e", e=4)

                mmax = ep.tile([128, NTT], F32, tag="mmax")
                emax = ep.tile([128, NTT, 4], F32, tag="emax")
                nc.vector.reduce_max(mmax, lg_m, axis=AX)
                nc.vector.reduce_max(emax, lg_e, axis=AX)

                expsrc = ep.tile([128, NTT, NE], F32, tag="expsrc")
                nc.vector.tensor_sub(
                    expsrc[:, :, 0:4], lg_m, mmax.to_broadcast([128, NTT, 4])
                )
                nc.vector.tensor_sub(
                    expsrc[:, :, 4:NE].rearrange("p g (m e) -> p g m e", e=4),
                    lg_e,
                    emax.to_broadcast([128, NTT, 4, 4]),
                )
                expv = ep.tile([128, NTT, NE], F32, tag="expv")
                nc.scalar.activation(expv, expsrc, EXP)

                msum = ep.tile([128, NTT], F32, tag="msum")
                esum = ep.tile([128, NTT, 4], F32, tag="esum")
                nc.vector.reduce_sum(msum, expv[:, :, 0:4], axis=AX)
                nc.vector.reduce_sum(
                    esum, expv[:, :, 4:NE].rearrange("p g (m e) -> p g m e", e=4),
                    axis=AX,
                )
                denom = ep.tile([128, NTT, 4], F32, tag="denom")
                nc.vector.tensor_mul(denom, esum, msum.to_broadcast([128, NTT, 4]))
                rden = ep.tile([128, NTT, 4], F32, tag="rden")
                nc.vector.reciprocal(rden, denom)
                coef = ep.tile([128, NTT, 4], F32, tag="coef")
                nc.vector.tensor_mul(coef, expv[:, :, 0:4], rden)

                comb = outp.tile([128, NTT, 16], F32, tag="comb")
                nc.vector.tensor_mul(
                    comb.rearrange("p g (m e) -> p g m e", e=4),
                    expv[:, :, 4:NE].rearrange("p g (m e) -> p g m e", e=4),
                    coef.to_broadcast([128, NTT, 4, 4]),
                )
                nc.scalar.dma_start(out=probs_v[q], in_=comb)

                maxv = ep.tile([128, NTT, 8], F32, tag="maxv")
                idx = outp.tile([128, NTT, 8], U32, tag="idx")
                for i in range(NTT):
                    nc.vector.max(out=maxv[:, i, :], in_=comb[:, i, :])
                    nc.vector.max_index(
                        out=idx[:, i, :], in_max=maxv[:, i, :], in_values=comb[:, i, :]
                    )

                wsum = ep.tile([128, NTT], F32, tag="wsum")
                nc.vector.reduce_sum(wsum, maxv[:, :, 0:4], axis=AX)
                nc.vector.tensor_scalar_add(wsum, wsum, 1e-8)
                rw = ep.tile([128, NTT], F32, tag="rw")
                nc.vector.reciprocal(rw, wsum)
                topw = outp.tile([128, NTT, 4], F32, tag="topw")
                nc.vector.tensor_mul(
                    topw, maxv[:, :, 0:4], rw.to_broadcast([128, NTT, 4])
                )
                nc.scalar.dma_start(out=topw_v[q], in_=topw)
                nc.scalar.dma_start(out=topi_v[q], in_=idx[:, :, 0:4])

    nc.compile()
    return nc


def _get_nc():
    if "nc" not in _CACHE:
        _CACHE["nc"] = _build()
    return _CACHE["nc"]


def _split_bf16(x32):
    """x32 (f32) -> (hi, lo) bf16 with hi + lo ~= x32 (~2^-18 rel)."""
    bf = ml_dtypes.bfloat16
    hi = x32.astype(bf)
    lo = (x32 - hi.astype(np.float32)).astype(bf)
    return hi, lo


def kernel(hidden_states, Wm, We):
    global LAST_RESULT
    nc = _get_nc()

    h = np.asarray(hidden_states, dtype=np.float32)
    W = np.concatenate(
        [np.asarray(Wm, dtype=np.float32), np.asarray(We, dtype=np.float32)], axis=0
    )  # [20, 1536]

    w_hi, w_lo = _split_bf16(W)
    wT = np.ascontiguousarray(
        np.concatenate([w_hi.T, w_lo.T], axis=1)
    )  # [1536, 40] bf16

    h_hi, h_lo = _split_bf16(h)

    in_maps = []
    for c in range(N_CORES):
        sl = slice(c * TLOC, (c + 1) * TLOC)
        in_maps.append(
            {
                "hiT": np.ascontiguousarray(h_hi[sl].T),
                "loT": np.ascontiguousarray(h_lo[sl].T),
                "wT": wT,
            }
        )

    res = run_bass_kernel_spmd(nc, in_maps, core_ids=list(range(N_CORES)))
    LAST_RESULT = res

    probs = np.concatenate([res.results[c]["o_probs"] for c in range(N_CORES)], axis=0)
    topw = np.concatenate([res.results[c]["o_topw"] for c in range(N_CORES)], axis=0)
    topi = np.concatenate(
        [res.results[c]["o_topi"] for c in range(N_CORES)], axis=0
    ).astype(np.int32)
    return probs, topw, topi


# revision 8
# speedup vs baseline: 1.3182x; 1.3182x over previous
"""Trainium2 Bass kernel for nn_CognitiveRouter (hierarchical MoE routing).

Computation (see reference):
    module_logits = h @ Wm.T      (T,4)
    expert_logits = h @ We.T      (T,16)
    module_probs  = softmax(module_logits)
    expert_probs  = softmax(expert_logits.reshape(T,4,4), axis=-1)
    combined      = (module_probs[:,:,None]*expert_probs).reshape(T,16)
    topw, topi    = top_k(combined, 4);  topw /= (sum(topw)+1e-8)

Strategy:
  - Data-parallel: shard T=32768 across 8 NeuronCores (4096 tokens each).
  - Host prep: W = [Wm;We] (20,1536) and h are split into bf16 hi/lo pairs
    (x = hi + lo captures ~2^-18 relative accuracy, fp32-class logits) and
    transposed so the contraction dim D lands on SBUF partitions.
    (hi + lo)@(Whi + Wlo) is computed with 2 matmul passes per k-tile into
    one PSUM accumulator of 40 columns [hi*Whi + lo*Whi | hi*Wlo + lo*Wlo],
    then logits = psum[:, :20] + psum[:, 20:40]  (all 4 cross terms).
  - Per core: 3 chunks of 1024 tokens + 2 tail chunks of 512 (finer tail
    granularity shortens the serial matmul+epilogue chain after the last
    DMA lands; big 3 MB slab DMAs elsewhere keep SDMA efficiency ~93%).
    Each chunk: [128,12,tsz] bf16 hi/lo slabs, token-tile matmuls
    (stationary = h tile, moving = W 40 cols), packed softmax + top-k
    epilogue. Top-4 via vector.max (top-8 sorted desc) + max_index
    (ties -> ascending distinct indices, matching jax.lax.top_k).
"""

import sys

if "/opt/trn_rl_repo" not in sys.path:
    sys.path.insert(0, "/opt/trn_rl_repo")

import ml_dtypes
import numpy as np

import concourse.bacc as bacc
import concourse.mybir as mybir
import concourse.tile as tile
from concourse.bass_utils import run_bass_kernel_spmd

N_CORES = 8
T, D = 32768, 1536
TLOC = T // N_CORES          # 4096 tokens per core
NKT = D // 128               # 12 k-tiles
NE = 20                      # 4 module + 16 expert logit columns
# (token_offset, tokens) chunks: big slabs early, fine granularity at the tail
CHUNKS = [(0, 1024), (1024, 1024), (2048, 1024), (3072, 512), (3584, 512)]

BF16 = mybir.dt.bfloat16
F32 = mybir.dt.float32
U32 = mybir.dt.uint32
AX = mybir.AxisListType.X
EXP = mybir.ActivationFunctionType.Exp
COPY = mybir.ActivationFunctionType.Copy

_CACHE = {}
LAST_RESULT = None  # BassKernelResults of the most recent run (for profiling)


def _build():
    nc = bacc.Bacc(trn_type="TRN2", target_bir_lowering=False, debug=False)

    hiT = nc.dram_tensor("hiT", [D, TLOC], BF16, kind="ExternalInput")
    loT = nc.dram_tensor("loT", [D, TLOC], BF16, kind="ExternalInput")
    wT = nc.dram_tensor("wT", [D, 2 * NE], BF16, kind="ExternalInput")
    o_probs = nc.dram_tensor("o_probs", [TLOC, 16], F32, kind="ExternalOutput")
    o_topw = nc.dram_tensor("o_topw", [TLOC, 4], F32, kind="ExternalOutput")
    o_topi = nc.dram_tensor("o_topi", [TLOC, 4], U32, kind="ExternalOutput")

    # DRAM views with the 128-partition dim on the left
    hiT_v = hiT.ap().rearrange("(k p) t -> p k t", p=128)   # [128, 12, 4096]
    loT_v = loT.ap().rearrange("(k p) t -> p k t", p=128)
    wT_v = wT.ap().rearrange("(k p) e -> p k e", p=128)     # [128, 12, 40]
    # token t_local = 128*n + p  ->  [128, 32, e]
    probs_v = o_probs.ap().rearrange("(n p) e -> p n e", p=128)
    topw_v = o_topw.ap().rearrange("(n p) e -> p n e", p=128)
    topi_v = o_topi.ap().rearrange("(n p) e -> p n e", p=128)

    with tile.TileContext(nc) as tc:
        with (
            tc.tile_pool(name="kt", bufs=3) as ktp,
            tc.tile_pool(name="wp", bufs=1) as wp,
            tc.tile_pool(name="ps", bufs=8, space="PSUM") as pp,
            tc.tile_pool(name="ep", bufs=2) as ep,
            tc.tile_pool(name="outp", bufs=3) as outp,
        ):
            w_sb = wp.tile([128, NKT, 2 * NE], BF16)
            nc.sync.dma_start(out=w_sb, in_=wT_v)

            for t0, tsz in CHUNKS:
                ntt = tsz // 128
                n0 = t0 // 128
                hi_sb = ktp.tile([128, NKT, 1024], BF16, tag="hi", name="hi_sb")
                lo_sb = ktp.tile([128, NKT, 1024], BF16, tag="lo", name="lo_sb")
                ts = slice(t0, t0 + tsz)
                nc.sync.dma_start(out=hi_sb[:, :, 0:tsz], in_=hiT_v[:, :, ts])
                nc.sync.dma_start(out=lo_sb[:, :, 0:tsz], in_=loT_v[:, :, ts])

                # ---- matmuls: per token tile, 24 accumulating matmuls ----
                ps_list = []
                for i in range(ntt):
                    ps = pp.tile([128, 2 * NE], F32, name="ps", tag="ps")
                    n_mm = 2 * NKT
                    j = 0
                    for k in range(NKT):
                        for src in (hi_sb, lo_sb):
                            nc.tensor.matmul(
                                ps,
                                lhsT=src[:, k, i * 128:(i + 1) * 128],
                                rhs=w_sb[:, k, :],
                                start=(j == 0),
                                stop=(j == n_mm - 1),
                            )
                            j += 1
                    ps_list.append(ps)

                # ---- epilogue: logits -> hierarchical softmax -> top-4 ----
                # PSUM -> SBUF on ScalarE (one PSUM read port; DVE cannot read
                # two PSUM operands), then a single SBUF-only add folds the
                # [hi*Whi+lo*Whi | hi*Wlo+lo*Wlo] halves.
                ps_sb = ep.tile([128, ntt, 2 * NE], F32, tag="ps_sb", name="ps_sb")
                for i in range(ntt):
                    nc.scalar.activation(ps_sb[:, i, :], ps_list[i], COPY)
                logits = ep.tile([128, ntt, NE], F32, tag="logits", name="logits")
                nc.vector.tensor_add(logits, ps_sb[:, :, 0:NE], ps_sb[:, :, NE:2 * NE])

                lg_m = logits[:, :, 0:4]
                lg_e = logits[:, :, 4:NE].rearrange("p g (m e) -> p g m e", e=4)

                mmax = ep.tile([128, ntt], F32, tag="mmax", name="mmax")
                emax = ep.tile([128, ntt, 4], F32, tag="emax", name="emax")
                nc.vector.reduce_max(mmax, lg_m, axis=AX)
                nc.vector.reduce_max(emax, lg_e, axis=AX)

                expsrc = ep.tile([128, ntt, NE], F32, tag="expsrc", name="expsrc")
                nc.vector.tensor_sub(
                    expsrc[:, :, 0:4], lg_m, mmax.to_broadcast([128, ntt, 4])
                )
                nc.vector.tensor_sub(
                    expsrc[:, :, 4:NE].rearrange("p g (m e) -> p g m e", e=4),
                    lg_e,
                    emax.to_broadcast([128, ntt, 4, 4]),
                )
                expv = ep.tile([128, ntt, NE], F32, tag="expv", name="expv")
                nc.scalar.activation(expv, expsrc, EXP)

                msum = ep.tile([128, ntt], F32, tag="msum", name="msum")
                esum = ep.tile([128, ntt, 4], F32, tag="esum", name="esum")
                nc.vector.reduce_sum(msum, expv[:, :, 0:4], axis=AX)
                nc.vector.reduce_sum(
                    esum, expv[:, :, 4:NE].rearrange("p g (m e) -> p g m e", e=4),
                    axis=AX,
                )
                denom = ep.tile([128, ntt, 4], F32, tag="denom", name="denom")
                nc.vector.tensor_mul(denom, esum, msum.to_broadcast([128, ntt, 4]))
                rden = ep.tile([128, ntt, 4], F32, tag="rden", name="rden")
                nc.vector.reciprocal(rden, denom)
                coef = ep.tile([128, ntt, 4], F32, tag="coef", name="coef")
                nc.vector.tensor_mul(coef, expv[:, :, 0:4], rden)

                comb = outp.tile([128, ntt, 16], F32, tag="comb", name="comb")
                nc.vector.tensor_mul(
                    comb.rearrange("p g (m e) -> p g m e", e=4),
                    expv[:, :, 4:NE].rearrange("p g (m e) -> p g m e", e=4),
                    coef.to_broadcast([128, ntt, 4, 4]),
                )
                nc.scalar.dma_start(out=probs_v[:, n0:n0 + ntt, :], in_=comb)

                maxv = ep.tile([128, ntt, 8], F32, tag="maxv", name="maxv")
                idx = outp.tile([128, ntt, 8], U32, tag="idx", name="idx")
                for i in range(ntt):
                    nc.vector.max(out=maxv[:, i, :], in_=comb[:, i, :])
                    nc.vector.max_index(
                        out=idx[:, i, :], in_max=maxv[:, i, :], in_values=comb[:, i, :]
                    )

                wsum = ep.tile([128, ntt], F32, tag="wsum", name="wsum")
                nc.vector.reduce_sum(wsum, maxv[:, :, 0:4], axis=AX)
                nc.vector.tensor_scalar_add(wsum, wsum, 1e-8)
                rw = ep.tile([128, ntt], F32, tag="rw", name="rw")
                nc.vector.reciprocal(rw, wsum)
                topw = outp.tile([128, ntt, 4], F32, tag="topw", name="topw")
                nc.vector.tensor_mul(
                    topw, maxv[:, :, 0:4], rw.to_broadcast([128, ntt, 4])
                )
                nc.scalar.dma_start(out=topw_v[:, n0:n0 + ntt, :], in_=topw)
                nc.scalar.dma_start(out=topi_v[:, n0:n0 + ntt, :], in_=idx[:, :, 0:4])

    nc.compile()
    return nc


def _get_nc():
    if "nc" not in _CACHE:
        _CACHE["nc"] = _build()
    return _CACHE["nc"]


def _split_bf16(x32):
    """x32 (f32) -> (hi, lo) bf16 with hi + lo ~= x32 (~2^-18 rel)."""
    bf = ml_dtypes.bfloat16
    hi = x32.astype(bf)
    lo = (x32 - hi.astype(np.float32)).astype(bf)
    return hi, lo


def kernel(hidden_states, Wm, We):
    global LAST_RESULT
    nc = _get_nc()

    h = np.asarray(hidden_states, dtype=np.float32)
    W = np.concatenate(
        [np.asarray(Wm, dtype=np.float32), np.asarray(We, dtype=np.float32)], axis=0
    )  # [20, 1536]

    w_hi, w_lo = _split_bf16(W)
    wT = np.ascontiguousarray(
        np.concatenate([w_hi.T, w_lo.T], axis=1)
    )  # [1536, 40] bf16

    h_hi, h_lo = _split_bf16(h)

    in_maps = []
    for c in range(N_CORES):
        sl = slice(c * TLOC, (c + 1) * TLOC)
        in_maps.append(
            {
                "hiT": np.ascontiguousarray(h_hi[sl].T),
                "loT": np.ascontiguousarray(h_lo[sl].T),
                "wT": wT,
            }
        )

    res = run_bass_kernel_spmd(nc, in_maps, core_ids=list(range(N_CORES)))
    LAST_RESULT = res

    probs = np.concatenate([res.results[c]["o_probs"] for c in range(N_CORES)], axis=0)
    topw = np.concatenate([res.results[c]["o_topw"] for c in range(N_CORES)], axis=0)
    topi = np.concatenate(
        [res.results[c]["o_topi"] for c in range(N_CORES)], axis=0
    ).astype(np.int32)
    return probs, topw, topi


# revision 12
# speedup vs baseline: 1.3920x; 1.0560x over previous
"""Trainium2 Bass kernel for nn_CognitiveRouter (hierarchical MoE routing).

Computation (see reference):
    module_logits = h @ Wm.T      (T,4)
    expert_logits = h @ We.T      (T,16)
    module_probs  = softmax(module_logits)
    expert_probs  = softmax(expert_logits.reshape(T,4,4), axis=-1)
    combined      = (module_probs[:,:,None]*expert_probs).reshape(T,16)
    topw, topi    = top_k(combined, 4);  topw /= (sum(topw)+1e-8)

Strategy:
  - Data-parallel: shard T=32768 across 8 NeuronCores (4096 tokens each).
  - Host prep: W = [Wm;We] (20,1536) and h are split into bf16 hi/lo pairs
    (x = hi + lo captures ~2^-18 relative accuracy, fp32-class logits) and
    transposed so the contraction dim D lands on SBUF partitions.
    (hi + lo)@(Whi + Wlo) is computed with 2 matmul passes per k-tile into
    one PSUM accumulator of 40 columns [hi*Whi + lo*Whi | hi*Wlo + lo*Wlo],
    then logits = psum[:, :20] + psum[:, 20:40]  (all 4 cross terms).
  - Per core: 3 chunks of 1024 tokens + 2 tail chunks of 512 (finer tail
    granularity shortens the serial matmul+epilogue chain after the last
    DMA lands; big 3 MB slab DMAs elsewhere keep SDMA efficiency ~93%).
    Each chunk: [128,12,tsz] bf16 hi/lo slabs, token-tile matmuls
    (stationary = h tile, moving = W 40 cols), packed softmax + top-k
    epilogue. Top-4 via vector.max (top-8 sorted desc) + max_index
    (ties -> ascending distinct indices, matching jax.lax.top_k).
"""

import sys

if "/opt/trn_rl_repo" not in sys.path:
    sys.path.insert(0, "/opt/trn_rl_repo")

import ml_dtypes
import numpy as np

import concourse.bacc as bacc
import concourse.mybir as mybir
import concourse.tile as tile
from concourse.bass_utils import run_bass_kernel_spmd

N_CORES = 8
T, D = 32768, 1536
TLOC = T // N_CORES          # 4096 tokens per core
NKT = D // 128               # 12 k-tiles
NE = 20                      # 4 module + 16 expert logit columns
QT = 1024                    # tokens per chunk (3 MB hi/lo slabs)
NQ = TLOC // QT              # 4 chunks
NTT = QT // 128              # 8 token tiles per chunk

BF16 = mybir.dt.bfloat16
F32 = mybir.dt.float32
U32 = mybir.dt.uint32
AX = mybir.AxisListType.X
EXP = mybir.ActivationFunctionType.Exp
COPY = mybir.ActivationFunctionType.Copy

_CACHE = {}
LAST_RESULT = None  # BassKernelResults of the most recent run (for profiling)


def _build():
    nc = bacc.Bacc(trn_type="TRN2", target_bir_lowering=False, debug=False)

    hiT = nc.dram_tensor("hiT", [D, TLOC], BF16, kind="ExternalInput")
    loT = nc.dram_tensor("loT", [D, TLOC], BF16, kind="ExternalInput")
    wT = nc.dram_tensor("wT", [D, 2 * NE], BF16, kind="ExternalInput")
    o_probs = nc.dram_tensor("o_probs", [TLOC, 16], F32, kind="ExternalOutput")
    o_topw = nc.dram_tensor("o_topw", [TLOC, 4], F32, kind="ExternalOutput")
    o_topi = nc.dram_tensor("o_topi", [TLOC, 4], U32, kind="ExternalOutput")

    # DRAM views with the 128-partition dim on the left
    hiT_v = hiT.ap().rearrange("(k p) t -> p k t", p=128)   # [128, 12, 4096]
    loT_v = loT.ap().rearrange("(k p) t -> p k t", p=128)
    wT_v = wT.ap().rearrange("(k p) e -> p k e", p=128)     # [128, 12, 40]
    # token t_local = 128*n + p  ->  [128, 32, e]
    probs_v = o_probs.ap().rearrange("(n p) e -> p n e", p=128)
    topw_v = o_topw.ap().rearrange("(n p) e -> p n e", p=128)
    topi_v = o_topi.ap().rearrange("(n p) e -> p n e", p=128)

    with tile.TileContext(nc) as tc:
        with (
            tc.tile_pool(name="kt", bufs=3) as ktp,
            tc.tile_pool(name="wp", bufs=1) as wp,
            tc.tile_pool(name="ps", bufs=8, space="PSUM") as pp,
            tc.tile_pool(name="ep", bufs=2) as ep,
            tc.tile_pool(name="outp", bufs=3) as outp,
        ):
            w_sb = wp.tile([128, NKT, 2 * NE], BF16)
            nc.sync.dma_start(out=w_sb, in_=wT_v)

            def emit_mms(ps_list, hi_sb, lo_sb, k0, nk, first, last, tiles):
                """Accumulating matmuls for k-tiles [k0, k0+nk) of the chunk."""
                for i in tiles:
                    for kk in range(nk):
                        for si, src in enumerate((hi_sb, lo_sb)):
                            nc.tensor.matmul(
                                ps_list[i],
                                lhsT=src[:, kk, i * 128:(i + 1) * 128],
                                rhs=w_sb[:, k0 + kk, :],
                                start=(first and kk == 0 and si == 0),
                                stop=(last and kk == nk - 1 and si == 1),
                            )

            def emit_epilogue(ps_list, n0, ntt):
                # ---- epilogue: logits -> hierarchical softmax -> top-4 ----
                # PSUM -> SBUF on ScalarE (one PSUM read port; DVE cannot read
                # two PSUM operands), then a single SBUF-only add folds the
                # [hi*Whi+lo*Whi | hi*Wlo+lo*Wlo] halves.
                ps_sb = ep.tile([128, ntt, 2 * NE], F32, tag="ps_sb", name="ps_sb")
                for i in range(ntt):
                    nc.scalar.activation(ps_sb[:, i, :], ps_list[i], COPY)
                logits = ep.tile([128, ntt, NE], F32, tag="logits", name="logits")
                nc.vector.tensor_add(logits, ps_sb[:, :, 0:NE], ps_sb[:, :, NE:2 * NE])

                lg_m = logits[:, :, 0:4]
                lg_e = logits[:, :, 4:NE].rearrange("p g (m e) -> p g m e", e=4)

                mmax = ep.tile([128, ntt], F32, tag="mmax", name="mmax")
                emax = ep.tile([128, ntt, 4], F32, tag="emax", name="emax")
                nc.vector.reduce_max(mmax, lg_m, axis=AX)
                nc.vector.reduce_max(emax, lg_e, axis=AX)

                expsrc = ep.tile([128, ntt, NE], F32, tag="expsrc", name="expsrc")
                nc.vector.tensor_sub(
                    expsrc[:, :, 0:4], lg_m, mmax.to_broadcast([128, ntt, 4])
                )
                nc.vector.tensor_sub(
                    expsrc[:, :, 4:NE].rearrange("p g (m e) -> p g m e", e=4),
                    lg_e,
                    emax.to_broadcast([128, ntt, 4, 4]),
                )
                expv = ep.tile([128, ntt, NE], F32, tag="expv", name="expv")
                nc.scalar.activation(expv, expsrc, EXP)

                msum = ep.tile([128, ntt], F32, tag="msum", name="msum")
                esum = ep.tile([128, ntt, 4], F32, tag="esum", name="esum")
                nc.vector.reduce_sum(msum, expv[:, :, 0:4], axis=AX)
                nc.vector.reduce_sum(
                    esum, expv[:, :, 4:NE].rearrange("p g (m e) -> p g m e", e=4),
                    axis=AX,
                )
                denom = ep.tile([128, ntt, 4], F32, tag="denom", name="denom")
                nc.vector.tensor_mul(denom, esum, msum.to_broadcast([128, ntt, 4]))
                rden = ep.tile([128, ntt, 4], F32, tag="rden", name="rden")
                nc.vector.reciprocal(rden, denom)
                coef = ep.tile([128, ntt, 4], F32, tag="coef", name="coef")
                nc.vector.tensor_mul(coef, expv[:, :, 0:4], rden)

                comb = outp.tile([128, ntt, 16], F32, tag="comb", name="comb")
                nc.vector.tensor_mul(
                    comb.rearrange("p g (m e) -> p g m e", e=4),
                    expv[:, :, 4:NE].rearrange("p g (m e) -> p g m e", e=4),
                    coef.to_broadcast([128, ntt, 4, 4]),
                )
                nc.scalar.dma_start(out=probs_v[:, n0:n0 + ntt, :], in_=comb)

                maxv = ep.tile([128, ntt, 8], F32, tag="maxv", name="maxv")
                idx = outp.tile([128, ntt, 8], U32, tag="idx", name="idx")
                for i in range(ntt):
                    nc.vector.max(out=maxv[:, i, :], in_=comb[:, i, :])
                    nc.vector.max_index(
                        out=idx[:, i, :], in_max=maxv[:, i, :], in_values=comb[:, i, :]
                    )

                wsum = ep.tile([128, ntt], F32, tag="wsum", name="wsum")
                nc.vector.reduce_sum(wsum, maxv[:, :, 0:4], axis=AX)
                nc.vector.tensor_scalar_add(wsum, wsum, 1e-8)
                rw = ep.tile([128, ntt], F32, tag="rw", name="rw")
                nc.vector.reciprocal(rw, wsum)
                topw = outp.tile([128, ntt, 4], F32, tag="topw", name="topw")
                nc.vector.tensor_mul(
                    topw, maxv[:, :, 0:4], rw.to_broadcast([128, ntt, 4])
                )
                nc.scalar.dma_start(out=topw_v[:, n0:n0 + ntt, :], in_=topw)
                nc.scalar.dma_start(out=topi_v[:, n0:n0 + ntt, :], in_=idx[:, :, 0:4])

            for q in range(NQ):
                ts = slice(q * QT, (q + 1) * QT)
                last_chunk = q == NQ - 1
                ps_list = [
                    pp.tile([128, 2 * NE], F32, name="ps", tag="ps")
                    for _ in range(NTT)
                ]
                if not last_chunk:
                    hi_sb = ktp.tile([128, NKT, QT], BF16, tag="hi", name="hi_sb")
                    lo_sb = ktp.tile([128, NKT, QT], BF16, tag="lo", name="lo_sb")
                    nc.sync.dma_start(out=hi_sb, in_=hiT_v[:, :, ts])
                    nc.sync.dma_start(out=lo_sb, in_=loT_v[:, :, ts])
                    emit_mms(ps_list, hi_sb, lo_sb, 0, NKT, True, True, range(NTT))
                    emit_epilogue(ps_list, q * NTT, NTT)
                else:
                    # tail chunk: k-halved DMAs (1.5 MB, still 2 KB runs) so
                    # the first half's matmuls overlap the second half's DMA;
                    # epilogue in two halves so softmax/top-k for tiles 0-3
                    # overlaps the remaining matmuls.
                    nkh = NKT // 2
                    for h in range(2):
                        hi_sb = ktp.tile(
                            [128, nkh, QT], BF16, tag=f"hit{h}", name="hi_sb", bufs=1
                        )
                        lo_sb = ktp.tile(
                            [128, nkh, QT], BF16, tag=f"lot{h}", name="lo_sb", bufs=1
                        )
                        ks = slice(h * nkh, (h + 1) * nkh)
                        nc.sync.dma_start(out=hi_sb, in_=hiT_v[:, ks, ts])
                        nc.sync.dma_start(out=lo_sb, in_=loT_v[:, ks, ts])
                        if h == 0:
                            emit_mms(ps_list, hi_sb, lo_sb, 0, nkh,
                                     True, False, range(NTT))
                        else:
                            emit_mms(ps_list, hi_sb, lo_sb, nkh, nkh,
                                     False, True, range(NTT // 2))
                            emit_epilogue(ps_list[: NTT // 2], q * NTT, NTT // 2)
                            emit_mms(ps_list, hi_sb, lo_sb, nkh, nkh,
                                     False, True, range(NTT // 2, NTT))
                            emit_epilogue(
                                ps_list[NTT // 2:], q * NTT + NTT // 2, NTT // 2
                            )

    nc.compile()
    return nc


def _get_nc():
    if "nc" not in _CACHE:
        _CACHE["nc"] = _build()
    return _CACHE["nc"]


def _split_bf16(x32):
    """x32 (f32) -> (hi, lo) bf16 with hi + lo ~= x32 (~2^-18 rel)."""
    bf = ml_dtypes.bfloat16
    hi = x32.astype(bf)
    lo = (x32 - hi.astype(np.float32)).astype(bf)
    return hi, lo


def kernel(hidden_states, Wm, We):
    global LAST_RESULT
    nc = _get_nc()

    h = np.asarray(hidden_states, dtype=np.float32)
    W = np.concatenate(
        [np.asarray(Wm, dtype=np.float32), np.asarray(We, dtype=np.float32)], axis=0
    )  # [20, 1536]

    w_hi, w_lo = _split_bf16(W)
    wT = np.ascontiguousarray(
        np.concatenate([w_hi.T, w_lo.T], axis=1)
    )  # [1536, 40] bf16

    h_hi, h_lo = _split_bf16(h)

    in_maps = []
    for c in range(N_CORES):
        sl = slice(c * TLOC, (c + 1) * TLOC)
        in_maps.append(
            {
                "hiT": np.ascontiguousarray(h_hi[sl].T),
                "loT": np.ascontiguousarray(h_lo[sl].T),
                "wT": wT,
            }
        )

    res = run_bass_kernel_spmd(nc, in_maps, core_ids=list(range(N_CORES)))
    LAST_RESULT = res

    probs = np.concatenate([res.results[c]["o_probs"] for c in range(N_CORES)], axis=0)
    topw = np.concatenate([res.results[c]["o_topw"] for c in range(N_CORES)], axis=0)
    topi = np.concatenate(
        [res.results[c]["o_topi"] for c in range(N_CORES)], axis=0
    ).astype(np.int32)
    return probs, topw, topi


# revision 13
# speedup vs baseline: 1.4201x; 1.0202x over previous
"""Trainium2 Bass kernel for nn_CognitiveRouter (hierarchical MoE routing).

Computation (see reference):
    module_logits = h @ Wm.T      (T,4)
    expert_logits = h @ We.T      (T,16)
    module_probs  = softmax(module_logits)
    expert_probs  = softmax(expert_logits.reshape(T,4,4), axis=-1)
    combined      = (module_probs[:,:,None]*expert_probs).reshape(T,16)
    topw, topi    = top_k(combined, 4);  topw /= (sum(topw)+1e-8)

Strategy:
  - Data-parallel: shard T=32768 across 8 NeuronCores (4096 tokens each).
  - Host prep: W = [Wm;We] (20,1536) and h are split into bf16 hi/lo pairs
    (x = hi + lo captures ~2^-18 relative accuracy, fp32-class logits) and
    transposed so the contraction dim D lands on SBUF partitions.
    (hi + lo)@(Whi + Wlo) is computed with 2 matmul passes per k-tile into
    one PSUM accumulator of 40 columns [hi*Whi + lo*Whi | hi*Wlo + lo*Wlo],
    then logits = psum[:, :20] + psum[:, 20:40]  (all 4 cross terms).
  - Per core: 3 chunks of 1024 tokens + 2 tail chunks of 512 (finer tail
    granularity shortens the serial matmul+epilogue chain after the last
    DMA lands; big 3 MB slab DMAs elsewhere keep SDMA efficiency ~93%).
    Each chunk: [128,12,tsz] bf16 hi/lo slabs, token-tile matmuls
    (stationary = h tile, moving = W 40 cols), packed softmax + top-k
    epilogue. Top-4 via vector.max (top-8 sorted desc) + max_index
    (ties -> ascending distinct indices, matching jax.lax.top_k).
"""

import sys

if "/opt/trn_rl_repo" not in sys.path:
    sys.path.insert(0, "/opt/trn_rl_repo")

import ml_dtypes
import numpy as np

import concourse.bacc as bacc
import concourse.mybir as mybir
import concourse.tile as tile
from concourse.bass_utils import run_bass_kernel_spmd

N_CORES = 8
T, D = 32768, 1536
TLOC = T // N_CORES          # 4096 tokens per core
NKT = D // 128               # 12 k-tiles
NE = 20                      # 4 module + 16 expert logit columns
QT = 1024                    # tokens per chunk (3 MB hi/lo slabs)
NQ = TLOC // QT              # 4 chunks
NTT = QT // 128              # 8 token tiles per chunk

BF16 = mybir.dt.bfloat16
F32 = mybir.dt.float32
U32 = mybir.dt.uint32
AX = mybir.AxisListType.X
EXP = mybir.ActivationFunctionType.Exp
COPY = mybir.ActivationFunctionType.Copy

_CACHE = {}
LAST_RESULT = None  # BassKernelResults of the most recent run (for profiling)


def _build():
    nc = bacc.Bacc(trn_type="TRN2", target_bir_lowering=False, debug=False)

    hiT = nc.dram_tensor("hiT", [D, TLOC], BF16, kind="ExternalInput")
    loT = nc.dram_tensor("loT", [D, TLOC], BF16, kind="ExternalInput")
    wT = nc.dram_tensor("wT", [D, 2 * NE], BF16, kind="ExternalInput")
    o_probs = nc.dram_tensor("o_probs", [TLOC, 16], F32, kind="ExternalOutput")
    o_topw = nc.dram_tensor("o_topw", [TLOC, 4], F32, kind="ExternalOutput")
    o_topi = nc.dram_tensor("o_topi", [TLOC, 4], U32, kind="ExternalOutput")

    # DRAM views with the 128-partition dim on the left
    hiT_v = hiT.ap().rearrange("(k p) t -> p k t", p=128)   # [128, 12, 4096]
    loT_v = loT.ap().rearrange("(k p) t -> p k t", p=128)
    wT_v = wT.ap().rearrange("(k p) e -> p k e", p=128)     # [128, 12, 40]
    # token t_local = 128*n + p  ->  [128, 32, e]
    probs_v = o_probs.ap().rearrange("(n p) e -> p n e", p=128)
    topw_v = o_topw.ap().rearrange("(n p) e -> p n e", p=128)
    topi_v = o_topi.ap().rearrange("(n p) e -> p n e", p=128)

    with tile.TileContext(nc) as tc:
        with (
            tc.tile_pool(name="kt", bufs=3) as ktp,
            tc.tile_pool(name="wp", bufs=1) as wp,
            tc.tile_pool(name="ps", bufs=8, space="PSUM") as pp,
            tc.tile_pool(name="ep", bufs=2) as ep,
            tc.tile_pool(name="outp", bufs=3) as outp,
        ):
            w_sb = wp.tile([128, NKT, 2 * NE], BF16)
            nc.sync.dma_start(out=w_sb, in_=wT_v)

            def emit_mms(ps_list, hi_sb, lo_sb, k0, nk, first, last, tiles):
                """Accumulating matmuls for k-tiles [k0, k0+nk) of the chunk."""
                for i in tiles:
                    for kk in range(nk):
                        for si, src in enumerate((hi_sb, lo_sb)):
                            nc.tensor.matmul(
                                ps_list[i],
                                lhsT=src[:, kk, i * 128:(i + 1) * 128],
                                rhs=w_sb[:, k0 + kk, :],
                                start=(first and kk == 0 and si == 0),
                                stop=(last and kk == nk - 1 and si == 1),
                            )

            def emit_epilogue(ps_list, n0, ntt):
                # ---- epilogue: logits -> hierarchical softmax -> top-4 ----
                # PSUM -> SBUF on ScalarE (one PSUM read port; DVE cannot read
                # two PSUM operands), then a single SBUF-only add folds the
                # [hi*Whi+lo*Whi | hi*Wlo+lo*Wlo] halves.
                ps_sb = ep.tile([128, ntt, 2 * NE], F32, tag="ps_sb", name="ps_sb")
                for i in range(ntt):
                    nc.scalar.activation(ps_sb[:, i, :], ps_list[i], COPY)
                logits = ep.tile([128, ntt, NE], F32, tag="logits", name="logits")
                nc.vector.tensor_add(logits, ps_sb[:, :, 0:NE], ps_sb[:, :, NE:2 * NE])

                lg_m = logits[:, :, 0:4]
                lg_e = logits[:, :, 4:NE].rearrange("p g (m e) -> p g m e", e=4)

                mmax = ep.tile([128, ntt], F32, tag="mmax", name="mmax")
                emax = ep.tile([128, ntt, 4], F32, tag="emax", name="emax")
                nc.vector.reduce_max(mmax, lg_m, axis=AX)
                nc.vector.reduce_max(emax, lg_e, axis=AX)

                expsrc = ep.tile([128, ntt, NE], F32, tag="expsrc", name="expsrc")
                nc.vector.tensor_sub(
                    expsrc[:, :, 0:4], lg_m, mmax.to_broadcast([128, ntt, 4])
                )
                nc.vector.tensor_sub(
                    expsrc[:, :, 4:NE].rearrange("p g (m e) -> p g m e", e=4),
                    lg_e,
                    emax.to_broadcast([128, ntt, 4, 4]),
                )
                expv = ep.tile([128, ntt, NE], F32, tag="expv", name="expv")
                nc.scalar.activation(expv, expsrc, EXP)

                msum = ep.tile([128, ntt], F32, tag="msum", name="msum")
                esum = ep.tile([128, ntt, 4], F32, tag="esum", name="esum")
                nc.vector.reduce_sum(msum, expv[:, :, 0:4], axis=AX)
                nc.vector.reduce_sum(
                    esum, expv[:, :, 4:NE].rearrange("p g (m e) -> p g m e", e=4),
                    axis=AX,
                )
                denom = ep.tile([128, ntt, 4], F32, tag="denom", name="denom")
                nc.vector.tensor_mul(denom, esum, msum.to_broadcast([128, ntt, 4]))
                rden = ep.tile([128, ntt, 4], F32, tag="rden", name="rden")
                nc.vector.reciprocal(rden, denom)
                coef = ep.tile([128, ntt, 4], F32, tag="coef", name="coef")
                nc.vector.tensor_mul(coef, expv[:, :, 0:4], rden)

                comb = outp.tile([128, ntt, 16], F32, tag="comb", name="comb")
                nc.vector.tensor_mul(
                    comb.rearrange("p g (m e) -> p g m e", e=4),
                    expv[:, :, 4:NE].rearrange("p g (m e) -> p g m e", e=4),
                    coef.to_broadcast([128, ntt, 4, 4]),
                )
                nc.scalar.dma_start(out=probs_v[:, n0:n0 + ntt, :], in_=comb)

                maxv = ep.tile([128, ntt, 8], F32, tag="maxv", name="maxv")
                idx = outp.tile([128, ntt, 8], U32, tag="idx", name="idx")
                for i in range(ntt):
                    nc.vector.max(out=maxv[:, i, :], in_=comb[:, i, :])
                    nc.vector.max_index(
                        out=idx[:, i, :], in_max=maxv[:, i, :], in_values=comb[:, i, :]
                    )

                wsum = ep.tile([128, ntt], F32, tag="wsum", name="wsum")
                nc.vector.reduce_sum(wsum, maxv[:, :, 0:4], axis=AX)
                nc.vector.tensor_scalar_add(wsum, wsum, 1e-8)
                rw = ep.tile([128, ntt], F32, tag="rw", name="rw")
                nc.vector.reciprocal(rw, wsum)
                topw = outp.tile([128, ntt, 4], F32, tag="topw", name="topw")
                nc.vector.tensor_mul(
                    topw, maxv[:, :, 0:4], rw.to_broadcast([128, ntt, 4])
                )
                nc.scalar.dma_start(out=topw_v[:, n0:n0 + ntt, :], in_=topw)
                nc.scalar.dma_start(out=topi_v[:, n0:n0 + ntt, :], in_=idx[:, :, 0:4])

            for q in range(NQ):
                ts = slice(q * QT, (q + 1) * QT)
                ps_list = [
                    pp.tile([128, 2 * NE], F32, name="ps", tag="ps")
                    for _ in range(NTT)
                ]
                hi_sb = ktp.tile([128, NKT, QT], BF16, tag="hi", name="hi_sb")
                lo_sb = ktp.tile([128, NKT, QT], BF16, tag="lo", name="lo_sb")
                nc.sync.dma_start(out=hi_sb, in_=hiT_v[:, :, ts])
                nc.sync.dma_start(out=lo_sb, in_=loT_v[:, :, ts])
                if q < NQ - 1:
                    emit_mms(ps_list, hi_sb, lo_sb, 0, NKT, True, True, range(NTT))
                    emit_epilogue(ps_list, q * NTT, NTT)
                else:
                    # last chunk: same DMA shape (the DMA FIFO stream is the
                    # critical resource — keep it identical), but emit the
                    # matmuls and epilogue in two token-tile halves so the
                    # first half's softmax/top-k overlaps the second half's
                    # matmuls, shortening the post-DMA serial tail.
                    h1, h2 = range(NTT // 2), range(NTT // 2, NTT)
                    emit_mms(ps_list, hi_sb, lo_sb, 0, NKT, True, True, h1)
                    emit_epilogue(ps_list[: NTT // 2], q * NTT, NTT // 2)
                    emit_mms(ps_list, hi_sb, lo_sb, 0, NKT, True, True, h2)
                    emit_epilogue(
                        ps_list[NTT // 2:], q * NTT + NTT // 2, NTT // 2
                    )

    nc.compile()
    return nc


def _get_nc():
    if "nc" not in _CACHE:
        _CACHE["nc"] = _build()
    return _CACHE["nc"]


def _split_bf16(x32):
    """x32 (f32) -> (hi, lo) bf16 with hi + lo ~= x32 (~2^-18 rel)."""
    bf = ml_dtypes.bfloat16
    hi = x32.astype(bf)
    lo = (x32 - hi.astype(np.float32)).astype(bf)
    return hi, lo


def kernel(hidden_states, Wm, We):
    global LAST_RESULT
    nc = _get_nc()

    h = np.asarray(hidden_states, dtype=np.float32)
    W = np.concatenate(
        [np.asarray(Wm, dtype=np.float32), np.asarray(We, dtype=np.float32)], axis=0
    )  # [20, 1536]

    w_hi, w_lo = _split_bf16(W)
    wT = np.ascontiguousarray(
        np.concatenate([w_hi.T, w_lo.T], axis=1)
    )  # [1536, 40] bf16

    h_hi, h_lo = _split_bf16(h)

    in_maps = []
    for c in range(N_CORES):
        sl = slice(c * TLOC, (c + 1) * TLOC)
        in_maps.append(
            {
                "hiT": np.ascontiguousarray(h_hi[sl].T),
                "loT": np.ascontiguousarray(h_lo[sl].T),
                "wT": wT,
            }
        )

    res = run_bass_kernel_spmd(nc, in_maps, core_ids=list(range(N_CORES)))
    LAST_RESULT = res

    probs = np.concatenate([res.results[c]["o_probs"] for c in range(N_CORES)], axis=0)
    topw = np.concatenate([res.results[c]["o_topw"] for c in range(N_CORES)], axis=0)
    topi = np.concatenate(
        [res.results[c]["o_topi"] for c in range(N_CORES)], axis=0
    ).astype(np.int32)
    return probs, topw, topi
